# revision 1
# baseline (speedup 1.0000x reference)
"""Trainium2 Bass kernel for nn_KATLayer (KAT basis-function layer).

out[b,o] = sum_{i,n} exp(-z^2) * (1 + erf(alpha*z/sqrt(2))) * w[i,o,n]
  z = (x[b,i] - c[i,o,n]) / (|sigma|+1e-8),  c = |scale|*mx_start + mx_train

Sharding: output dim O split across 8 cores (O_shard=64). Per core:
  partitions = i (4 chunks of 128), free = (o_local, n) = 1024 per tile,
  one tile per (b, i_chunk) = 128 tiles.

Per tile:
  DVE:  zm = (c - x)*rinv          [scalar_tensor_tensor, x per-partition]
        um = (c - x)*A             [A = alpha*rinv/sqrt(2)]
  ACT:  e  = Derivative_Erf(zm)    [= 2/sqrt(pi)*exp(-z^2), even in z]
        t  = Erf(-um)              [= erf(alpha*z/sqrt(2))]
  DVE:  s  = (t + 1)*wt            [wt = w*sqrt(pi)/2]
        p  = e*s
  PE :  psum[b, half] += onehot_b.T @ p_half   (reduces over i-partitions)
Final: DVE reduce over n: psum(32,64,16) -> (32,64); DMA out.

Derivative_Erf and Erf live in different ACT table sets (~2.7us/switch), so
tiles are processed in groups with phase-batched activations (2 switches per
group). Phase order within a group keeps zm and um lifetimes disjoint so
both phases share one SBUF pool.

fp16 variant (default): e/t/s/p and the PE-reduce stream run in fp16
(DVE 2x mode, PE 1 cyc/row); zm/um stay fp32 (z precision is critical).
"""
import sys

sys.path.insert(0, "/opt/trn_rl_repo")
import math

import numpy as np

B, I, O, N = 32, 512, 512, 16
NCORES = 8
OS = O // NCORES          # 64 output dims per core
KC = I // 128             # 4 i-chunks
P = 128
G = 12                    # tiles per activation-phase group
INV_SQRT2 = 0.7071067811865476
SQRT_PI_2 = math.sqrt(math.pi) / 2.0

_CACHE = {}
LAST_RESULTS = None


def _build_nc(reps=1, fp16_products=True, G=G):
    import concourse.bacc as bacc
    import concourse.mybir as mybir
    from concourse import tile

    fp32 = mybir.dt.float32
    fp16 = mybir.dt.float16
    pdt = fp16 if fp16_products else fp32
    AF = mybir.ActivationFunctionType
    ALU = mybir.AluOpType

    nc = bacc.Bacc(
        "TRN2", target_bir_lowering=False, debug=False, num_devices=NCORES
    )
    c_d = nc.dram_tensor("c", [KC, P, OS, N], fp32, kind="ExternalInput")
    r_d = nc.dram_tensor("r", [KC, P, OS, N], fp32, kind="ExternalInput")
    a_d = nc.dram_tensor("a", [KC, P, OS, N], fp32, kind="ExternalInput")
    w_d = nc.dram_tensor("w", [KC, P, OS, N], pdt, kind="ExternalInput")
    x_d = nc.dram_tensor("x", [KC, P, B], fp32, kind="ExternalInput")
    oh_d = nc.dram_tensor("oh", [P, B, B], pdt, kind="ExternalInput")
    out_d = nc.dram_tensor("out", [B, OS], fp32, kind="ExternalOutput")

    with tile.TileContext(nc) as tc:
        with (
            tc.tile_pool(name="const", bufs=1) as cpool,
            tc.tile_pool(name="zu", bufs=G + 2) as zup,
            tc.tile_pool(name="et", bufs=2 * G + 3) as etp,
            tc.tile_pool(name="sp", bufs=3) as spool,
            tc.tile_pool(name="pp", bufs=3) as ppool,
            tc.tile_pool(name="psum", bufs=1, space="PSUM") as psp,
            tc.tile_pool(name="outp", bufs=1) as opool,
        ):
            c_sb, r_sb, a_sb, w_sb = [], [], [], []
            for k in range(KC):
                for lst, dram, nm, dt_ in (
                    (c_sb, c_d, "c", fp32),
                    (r_sb, r_d, "r", fp32),
                    (a_sb, a_d, "a", fp32),
                    (w_sb, w_d, "w", pdt),
                ):
                    t = cpool.tile([P, OS, N], dt_, tag=f"{nm}{k}")
                    nc.sync.dma_start(t[:], dram[k])
                    lst.append(t)
            x_sb = cpool.tile([P, KC * B], fp32, tag="x")
            for k in range(KC):
                nc.sync.dma_start(x_sb[:, k * B : (k + 1) * B], x_d[k])
            oh_sb = cpool.tile([P, B, B], pdt, tag="oh")
            nc.sync.dma_start(oh_sb[:], oh_d[:])

            psum_t = psp.tile([B, OS, N], fp32)
            tiles = [(b, k) for b in range(B) for k in range(KC)]
            out_sb = opool.tile([B, OS], fp32)

            for rep in range(reps):
                n_mm = 0
                for g0 in range(0, len(tiles), G):
                    grp = tiles[g0 : g0 + G]
                    # phase 1a: all zm; phase 1b: DerivErf(zm) -> e (fp16)
                    es, ts_ = [], []
                    for b, k in grp:
                        xcol = x_sb[:, k * B + b : k * B + b + 1]
                        zm = zup.tile([P, OS, N], fp32, tag="zu")
                        nc.vector.scalar_tensor_tensor(
                            zm[:], c_sb[k][:], xcol, r_sb[k][:],
                            op0=ALU.subtract, op1=ALU.mult,
                        )
                        e = etp.tile([P, OS, N], pdt, tag="et")
                        es.append((zm, e))
                    for zm, e in es:
                        nc.scalar.activation(e[:], zm[:], AF.Derivative_Erf)
                    # phase 2a: all um; phase 2b: Erf(-um) -> t (fp16)
                    for b, k in grp:
                        xcol = x_sb[:, k * B + b : k * B + b + 1]
                        um = zup.tile([P, OS, N], fp32, tag="zu")
                        nc.vector.scalar_tensor_tensor(
                            um[:], c_sb[k][:], xcol, a_sb[k][:],
                            op0=ALU.subtract, op1=ALU.mult,
                        )
                        t_ = etp.tile([P, OS, N], pdt, tag="et")
                        ts_.append((um, t_))
                    for um, t_ in ts_:
                        nc.scalar.activation(t_[:], um[:], AF.Erf, scale=-1.0)
                    # phase 3: s = (t+1)*wt ; p = e*s ; PE-reduce over i
                    for (b, k), (zm, e), (um, t_) in zip(grp, es, ts_):
                        s = spool.tile([P, OS, N], pdt)
                        nc.vector.scalar_tensor_tensor(
                            s[:], t_[:], 1.0, w_sb[k][:],
                            op0=ALU.add, op1=ALU.mult,
                        )
                        p = ppool.tile([P, OS, N], pdt)
                        nc.vector.tensor_mul(p[:], e[:], s[:])
                        for h in range(2):
                            nc.tensor.matmul(
                                psum_t[:, 32 * h : 32 * (h + 1), :],
                                oh_sb[:, b, :],
                                p[:, 32 * h : 32 * (h + 1), :],
                                start=(n_mm < 2),
                                stop=(n_mm >= 2 * len(tiles) - 2),
                            )
                            n_mm += 1

                nc.vector.tensor_reduce(
                    out_sb[:], psum_t[:], axis=mybir.AxisListType.X, op=ALU.add
                )
            nc.sync.dma_start(out_d[:], out_sb[:])

    nc.compile()
    return nc


def _prep_inputs(x, mx_train, scale, sigma, alpha, w, mx_start,
                 fp16_products=True):
    pdt = np.float16 if fp16_products else np.float32
    c = (np.abs(scale)[:, :, None] * mx_start[None, None, :]
         + mx_train[:, :, None]).astype(np.float32)
    rinv = (1.0 / (np.abs(sigma) + 1e-8)).astype(np.float32)
    A = (alpha * INV_SQRT2 * rinv).astype(np.float32)
    wt = (w * SQRT_PI_2).astype(pdt)
    xT = np.ascontiguousarray(x.T.reshape(KC, P, B)).astype(np.float32)
    oh = np.broadcast_to(np.eye(B, dtype=pdt), (P, B, B))
    oh = np.ascontiguousarray(oh)

    in_maps = []
    for d in range(NCORES):
        sl = slice(d * OS, (d + 1) * OS)
        in_maps.append({
            "c": np.ascontiguousarray(c[:, sl].reshape(KC, P, OS, N)),
            "r": np.ascontiguousarray(rinv[:, sl].reshape(KC, P, OS, N)),
            "a": np.ascontiguousarray(A[:, sl].reshape(KC, P, OS, N)),
            "w": np.ascontiguousarray(wt[:, sl].reshape(KC, P, OS, N)),
            "x": xT,
            "oh": oh,
        })
    return in_maps


def kernel(x, mx_train, scale, sigma, alpha, w, mx_start, _trace=False):
    global LAST_RESULTS
    from concourse.bass_utils import run_bass_kernel_spmd

    if "nc" not in _CACHE:
        _CACHE["nc"] = _build_nc()
    nc = _CACHE["nc"]
    in_maps = _prep_inputs(
        np.asarray(x, np.float32), np.asarray(mx_train, np.float32),
        np.asarray(scale, np.float32), np.asarray(sigma, np.float32),
        np.asarray(alpha, np.float32), np.asarray(w, np.float32),
        np.asarray(mx_start, np.float32),
    )
    res = run_bass_kernel_spmd(nc, in_maps, core_ids=list(range(NCORES)),
                               trace=_trace)
    LAST_RESULTS = res
    return np.concatenate([r["out"] for r in res.results], axis=1)



# revision 2
# speedup vs baseline: 1.3599x; 1.3599x over previous
"""Trainium2 Bass kernel for nn_KATLayer (KAT basis-function layer).

out[b,o] = sum_{i,n} exp(-z^2) * (1 + erf(alpha*z/sqrt(2))) * w[i,o,n]
  z = (x[b,i] - c[i,o,n]) / (|sigma|+1e-8),  c = |scale|*mx_start + mx_train

Sharding: output dim O split across 8 cores (O_shard=64). Per core:
  partitions = i (4 chunks of 128), free = (o_local, n) = 1024 per tile,
  tiles processed in PAIRS (2 consecutive b, same i-chunk) so the
  elementwise/activation ops run at free=2048 to amortize fixed overheads.

Math restructure vs the naive pipeline (all products fp16, z kept fp16 —
validated 5.2e-4 rel err vs 2e-2 gate):
  DVE:  zm = (c - x)*rinv          [STT fp32-in -> fp16 out; = -z]
  ACT:  e  = Derivative_Erf(zm)    [= 2/sqrt(pi)*exp(-z^2), even in z]
  DVE:  q  = e*wt                  [wt = w*sqrt(pi)/2 -> q = w*exp(-z^2)]
  DVE:  um = zm*A3                 [A3 = alpha/sqrt(2); fp16 TT 2x mode]
  ACT:  t  = Erf(-um)              [= erf(alpha*z/sqrt(2))]
  DVE:  r  = q*t
  PE :  psum += onehot_b.T @ q ; psum += onehot_b.T @ r
        (the "+1" of (1+erf) is absorbed by accumulating BOTH the q and r
        streams in PSUM, killing the fp16 STT (1x-only uop) of the old
        s=(t+1)*wt formulation)
Final: DVE reduce over n: psum(32,64,16) -> (32,64); DMA out.

um/q/r run as fp16 tensor_tensor (2x mode, 2 elem/cyc/lane); per-k consts
(A3, wt) are read through stride-0 broadcast APs across the pair dim.

Derivative_Erf and Erf live in different ACT table sets (~2.7us/switch), so
pairs are processed in groups with phase-batched activations (2 switches per
group).
"""
import sys

sys.path.insert(0, "/opt/trn_rl_repo")
import math

import numpy as np

B, I, O, N = 32, 512, 512, 16
NCORES = 8
OS = O // NCORES          # 64 output dims per core
KC = I // 128             # 4 i-chunks
P = 128
GQ = 6                    # pairs per activation-phase group (12 tiles)
INV_SQRT2 = 0.7071067811865476
SQRT_PI_2 = math.sqrt(math.pi) / 2.0

_CACHE = {}
LAST_RESULTS = None


def _build_nc(reps=1, GQ=GQ):
    import concourse.bacc as bacc
    import concourse.mybir as mybir
    from concourse import tile

    fp32 = mybir.dt.float32
    fp16 = mybir.dt.float16
    AF = mybir.ActivationFunctionType
    ALU = mybir.AluOpType

    nc = bacc.Bacc(
        "TRN2", target_bir_lowering=False, debug=False, num_devices=NCORES
    )
    c_d = nc.dram_tensor("c", [KC, P, OS, N], fp32, kind="ExternalInput")
    r_d = nc.dram_tensor("r", [KC, P, OS, N], fp32, kind="ExternalInput")
    a_d = nc.dram_tensor("a", [KC, P, OS, N], fp16, kind="ExternalInput")
    w_d = nc.dram_tensor("w", [KC, P, OS, N], fp16, kind="ExternalInput")
    x_d = nc.dram_tensor("x", [KC, P, B], fp32, kind="ExternalInput")
    oh_d = nc.dram_tensor("oh", [P, B, B], fp16, kind="ExternalInput")
    out_d = nc.dram_tensor("out", [B, OS], fp32, kind="ExternalOutput")

    with tile.TileContext(nc) as tc:
        with (
            tc.tile_pool(name="const", bufs=1) as cpool,
            tc.tile_pool(name="zp", bufs=GQ + 2) as zpool,
            tc.tile_pool(name="qp", bufs=GQ + 2) as qpool,
            tc.tile_pool(name="tp", bufs=GQ + 2) as tpool,
            tc.tile_pool(name="ep", bufs=3) as epool,
            tc.tile_pool(name="up", bufs=3) as upool,
            tc.tile_pool(name="rp", bufs=3) as rpool,
            tc.tile_pool(name="psum", bufs=1, space="PSUM") as psp,
            tc.tile_pool(name="outp", bufs=1) as opool,
        ):
            c_sb, r_sb, a_sb, w_sb = [], [], [], []
            for k in range(KC):
                for lst, dram, nm, dt_ in (
                    (c_sb, c_d, "c", fp32),
                    (r_sb, r_d, "r", fp32),
                    (a_sb, a_d, "a", fp16),
                    (w_sb, w_d, "w", fp16),
                ):
                    t = cpool.tile([P, OS, N], dt_, tag=f"{nm}{k}")
                    nc.sync.dma_start(t[:], dram[k])
                    lst.append(t)
            x_sb = cpool.tile([P, KC * B], fp32, tag="x")
            for k in range(KC):
                nc.sync.dma_start(x_sb[:, k * B : (k + 1) * B], x_d[k])
            oh_sb = cpool.tile([P, B, B], fp16, tag="oh")
            nc.sync.dma_start(oh_sb[:], oh_d[:])

            psum_t = psp.tile([B, OS, N], fp32)
            # pairs: 2 consecutive b, same k
            pairs = [(k, b) for k in range(KC) for b in range(0, B, 2)]
            n_pairs = len(pairs)
            out_sb = opool.tile([B, OS], fp32)

            def bcast2(t):
                return t[:, None].broadcast_to((P, 2, OS, N))

            for rep in range(reps):
                n_mm = 0
                total_mm = 8 * n_pairs
                for g0 in range(0, n_pairs, GQ):
                    grp = pairs[g0 : g0 + GQ]
                    # phase 1: zm for all pairs in group (DVE STT, fp32->fp16)
                    zms = []
                    for k, b in grp:
                        zm = zpool.tile([P, 2, OS, N], fp16, tag="zp")
                        for j in range(2):
                            xcol = x_sb[:, k * B + b + j : k * B + b + j + 1]
                            nc.vector.scalar_tensor_tensor(
                                zm[:, j], c_sb[k][:], xcol, r_sb[k][:],
                                op0=ALU.subtract, op1=ALU.mult,
                            )
                        zms.append(zm)
                    # phase 2: e = D_ERF(zm) (ACT, table A); q = e*wt (DVE)
                    qs = []
                    for (k, b), zm in zip(grp, zms):
                        e = epool.tile([P, 2, OS, N], fp16)
                        nc.scalar.activation(e[:], zm[:], AF.Derivative_Erf)
                        q = qpool.tile([P, 2, OS, N], fp16, tag="qp")
                        nc.vector.tensor_mul(q[:], e[:], bcast2(w_sb[k]))
                        qs.append(q)
                    # phase 3: um = zm*A3 (DVE); t = Erf(-um) (ACT, table B)
                    ts_ = []
                    for (k, b), zm in zip(grp, zms):
                        um = upool.tile([P, 2, OS, N], fp16)
                        nc.vector.tensor_mul(um[:], zm[:], bcast2(a_sb[k]))
                        t_ = tpool.tile([P, 2, OS, N], fp16, tag="tp")
                        nc.scalar.activation(t_[:], um[:], AF.Erf, scale=-1.0)
                        ts_.append(t_)
                    # phase 4: r = q*t (DVE); accumulate q and r streams (PE)
                    for (k, b), q, t_ in zip(grp, qs, ts_):
                        r_ = rpool.tile([P, 2, OS, N], fp16)
                        nc.vector.tensor_mul(r_[:], q[:], t_[:])
                        for j in range(2):
                            for src in (q, r_):
                                for h in range(2):
                                    nc.tensor.matmul(
                                        psum_t[:, 32 * h : 32 * (h + 1), :],
                                        oh_sb[:, b + j, :],
                                        src[:, j, 32 * h : 32 * (h + 1), :],
                                        start=(n_mm < 2),
                                        stop=(n_mm >= total_mm - 2),
                                    )
                                    n_mm += 1

                nc.vector.tensor_reduce(
                    out_sb[:], psum_t[:], axis=mybir.AxisListType.X, op=ALU.add
                )
            nc.sync.dma_start(out_d[:], out_sb[:])

    nc.compile()
    return nc


def _prep_inputs(x, mx_train, scale, sigma, alpha, w, mx_start):
    c = (np.abs(scale)[:, :, None] * mx_start[None, None, :]
         + mx_train[:, :, None]).astype(np.float32)
    rinv = (1.0 / (np.abs(sigma) + 1e-8)).astype(np.float32)
    A3 = (alpha * INV_SQRT2).astype(np.float16)
    wt = (w * SQRT_PI_2).astype(np.float16)
    xT = np.ascontiguousarray(x.T.reshape(KC, P, B)).astype(np.float32)
    oh = np.broadcast_to(np.eye(B, dtype=np.float16), (P, B, B))
    oh = np.ascontiguousarray(oh)

    in_maps = []
    for d in range(NCORES):
        sl = slice(d * OS, (d + 1) * OS)
        in_maps.append({
            "c": np.ascontiguousarray(c[:, sl].reshape(KC, P, OS, N)),
            "r": np.ascontiguousarray(rinv[:, sl].reshape(KC, P, OS, N)),
            "a": np.ascontiguousarray(A3[:, sl].reshape(KC, P, OS, N)),
            "w": np.ascontiguousarray(wt[:, sl].reshape(KC, P, OS, N)),
            "x": xT,
            "oh": oh,
        })
    return in_maps


def kernel(x, mx_train, scale, sigma, alpha, w, mx_start, _trace=False):
    global LAST_RESULTS
    from concourse.bass_utils import run_bass_kernel_spmd

    if "nc" not in _CACHE:
        _CACHE["nc"] = _build_nc()
    nc = _CACHE["nc"]
    in_maps = _prep_inputs(
        np.asarray(x, np.float32), np.asarray(mx_train, np.float32),
        np.asarray(scale, np.float32), np.asarray(sigma, np.float32),
        np.asarray(alpha, np.float32), np.asarray(w, np.float32),
        np.asarray(mx_start, np.float32),
    )
    res = run_bass_kernel_spmd(nc, in_maps, core_ids=list(range(NCORES)),
                               trace=_trace)
    LAST_RESULTS = res
    return np.concatenate([r["out"] for r in res.results], axis=1)
